# revision 22
# baseline (speedup 1.0000x reference)
"""Directional GraphSAGE (3-layer) Trainium2 kernel, 8-core SPMD.

Strategy:
  - Nodes sharded contiguously across 8 cores (N/8 per core).
  - fwd aggregation (segment-sum over dst) partitioned by dst owner;
    bwd aggregation (segment-sum over src) partitioned by src owner.
  - Per core, its node range is split into 256-node GROUPS. Each group's
    incoming edges are gathered (fp16 rows, indirect DMA from an HBM table)
    in 128-edge tiles; a scaled one-hot matrix [128 edges x 256 dst] is built
    on the vector engine (one tensor_scalar: (iota == dst_local) * val) and
    the TensorEngine accumulates  msgs^T @ onehot  ->  PSUM [128 feat, 256]
    (feature-major aggregation, fp32 accumulate).
  - Linear layer: out_T[fo, n] = Wx^T x_T + Wf^T fwd_T + Wb^T bwd_T in PSUM,
    bias+ReLU on the scalar engine. Everything stays feature-major; the x
    (self) path stays fp32 end-to-end.
  - New features are PE-transposed to node-major fp16 rows, DMA'd to a
    per-core shard buffer, and AllGather'd into the next layer's gather table.
  - All data-dependent constants (per-group tile counts) are maxed across
    cores so the single SPMD program is structurally identical on all cores;
    per-core differences live in input arrays (gather indices, dst slots,
    edge values).
"""

import os
import sys

import numpy as np

sys.path.insert(0, "/opt/trn_rl_repo")

NCORES = int(os.environ.get("KERNEL_NCORES", "8"))
SKIP_COLLECTIVE = os.environ.get("KERNEL_SKIP_COLLECTIVE", "0") == "1"
P = 128            # partitions / feature dim tile
GROUP = 256        # dst nodes covered by one PSUM accumulation group
SB = 512           # nodes per superblock (2 groups)


# ----------------------------------------------------------------------------
# Host-side prep
# ----------------------------------------------------------------------------

def _prep_direction(gather_node, out_node, val, N, shard, lo_cap):
    """Build per-core gather/onehot grids for one aggregation direction.

    For edge e: contributes val[e] * x[gather_node[e]] to output row
    out_node[e].  Partitioned by out_node's owning core; within a core,
    grouped by 256-node groups of the local node range; within a group,
    split into lo (gather row < lo_cap) and hi halves so int16 gather
    indices stay in range.

    Tile layout per (group, half): edge slot i -> (partition i%128,
    tile i//128), matching dma_gather's output placement.  dst/val grids
    [128, sum(T)] hold per-tile onehot scalars (column = one tile); the
    idx grid [128, 8*sum(T)] holds int16 indices in dma_gather's
    16-partition-wrapped layout replicated across the 8 Q7 cores.
    """
    ngroups = (shard + GROUP - 1) // GROUP
    owner = out_node // shard
    local = out_node - owner * shard
    group = local // GROUP
    dst_local = (local - group * GROUP).astype(np.float32)
    is_hi = gather_node >= lo_cap

    # counts[c, g, half]
    counts = np.zeros((NCORES, ngroups, 2), dtype=np.int64)
    np.add.at(counts, (owner, group, is_hi.astype(np.int64)), 1)
    cmax = counts.max(axis=0)  # [ngroups, 2]
    TA = ((cmax[:, 0] + P - 1) // P).astype(np.int64)
    TB = ((cmax[:, 1] + P - 1) // P).astype(np.int64)
    TA = np.maximum(TA, (TB == 0).astype(np.int64))  # >=1 tile per group
    Tsum = TA + TB
    offs = np.concatenate([[0], np.cumsum(Tsum)]).astype(np.int64)
    Tt = int(offs[-1])

    idx_g = np.zeros((NCORES, P, 8 * Tt), dtype=np.int16)
    dst_g = np.zeros((NCORES, P, Tt), dtype=np.float32)
    val_g = np.zeros((NCORES, P, Tt), dtype=np.float32)

    # sort edges by (owner, group, half) once
    key = (owner * ngroups + group) * 2 + is_hi
    order = np.argsort(key, kind="stable")
    ks = key[order]
    gn = gather_node[order]
    dl = dst_local[order]
    vv = val[order]
    nkey = NCORES * ngroups * 2
    starts = np.searchsorted(ks, np.arange(nkey))
    ends = np.searchsorted(ks, np.arange(nkey) + 1)

    def fill(c, col0, t, rows, dlocal, vals):
        """Place a padded run of t tiles at dv-columns [col0, col0+t)."""
        cap = P * t
        n = len(rows)
        assert n <= cap
        bi = np.zeros(cap, dtype=np.int16)
        bd = np.zeros(cap, dtype=np.float32)
        bv = np.zeros(cap, dtype=np.float32)
        bi[:n] = rows.astype(np.int16)
        bd[:n] = dlocal
        bv[:n] = vals
        dst_g[c, :, col0:col0 + t] = bd.reshape(t, P).T
        val_g[c, :, col0:col0 + t] = bv.reshape(t, P).T
        # wrapped idx: element j at (j%16, j//16), replicated x8
        wrapped = bi.reshape(cap // 16, 16).T  # [16, 8t]
        idx_g[c, :, 8 * col0:8 * (col0 + t)] = np.tile(wrapped, (8, 1))

    for c in range(NCORES):
        for g in range(ngroups):
            kbase = (c * ngroups + g) * 2
            sA, eA = starts[kbase], ends[kbase]
            sB, eB = starts[kbase + 1], ends[kbase + 1]
            tA, tB = int(TA[g]), int(TB[g])
            col0 = int(offs[g])
            if tA > 0:
                fill(c, col0, tA, gn[sA:eA], dl[sA:eA], vv[sA:eA])
            if tB > 0:
                fill(c, col0 + tA, tB, gn[sB:eB] - lo_cap, dl[sB:eB],
                     vv[sB:eB])
    return TA, TB, offs, idx_g, dst_g, val_g


def _host_prep(x, W0, b0, W1, b1, W2, b2, src, dst, lo_cap):
    N, D = x.shape
    assert D == P
    assert N % NCORES == 0, N
    shard = N // NCORES

    deg_out = np.bincount(src, minlength=N).astype(np.float32)
    deg_in = np.bincount(dst, minlength=N).astype(np.float32)
    inv_out = 1.0 / np.maximum(deg_out, 1.0)
    inv_in = 1.0 / np.maximum(deg_in, 1.0)

    # fwd: out row = dst, gather src, scale inv_out[src]
    fTA, fTB, foffs, fidx, fdst, fval = _prep_direction(
        src.astype(np.int64), dst.astype(np.int64), inv_out[src], N, shard,
        lo_cap)
    # bwd: out row = src, gather dst, scale inv_in[dst]
    bTA, bTB, boffs, bidx, bdst, bval = _prep_direction(
        dst.astype(np.int64), src.astype(np.int64), inv_in[dst], N, shard,
        lo_cap)

    ngroups = (shard + GROUP - 1) // GROUP
    nsb = (ngroups + 1) // 2
    npad = nsb * SB

    x16 = np.ascontiguousarray(x.astype(np.float16))
    xT = np.zeros((NCORES, P, npad), dtype=np.float32)
    for c in range(NCORES):
        xT[c, :, :shard] = x[c * shard:(c + 1) * shard].T

    iota = np.tile(np.arange(GROUP, dtype=np.float16), (P, 1))

    Ws, bs = [], []
    for W, b in ((W0, b0), (W1, b1), (W2, b2)):
        Ws.append((np.ascontiguousarray(W[0:P]).astype(np.float32),
                   np.ascontiguousarray(W[P:2 * P]).astype(np.float16),
                   np.ascontiguousarray(W[2 * P:3 * P]).astype(np.float16)))
        bs.append(b.reshape(P, 1).astype(np.float32))

    meta = dict(N=N, shard=shard, ngroups=ngroups, nsb=nsb, npad=npad,
                lo_cap=lo_cap, fTA=fTA, fTB=fTB, foffs=foffs,
                bTA=bTA, bTB=bTB, boffs=boffs)
    per_core = []
    for c in range(NCORES):
        m = {
            "x16": x16,
            "xT0": np.ascontiguousarray(xT[c]),
            "fidx": np.ascontiguousarray(fidx[c]),
            "fdst": np.ascontiguousarray(fdst[c]),
            "fval": np.ascontiguousarray(fval[c]),
            "bidx": np.ascontiguousarray(bidx[c]),
            "bdst": np.ascontiguousarray(bdst[c]),
            "bval": np.ascontiguousarray(bval[c]),
            "iota": iota,
        }
        for li in range(3):
            m[f"Wx{li}"] = Ws[li][0]
            m[f"Wf{li}"] = Ws[li][1]
            m[f"Wb{li}"] = Ws[li][2]
            m[f"bias{li}"] = bs[li]
        per_core.append(m)
    return meta, per_core


# ----------------------------------------------------------------------------
# Bass program
# ----------------------------------------------------------------------------

def _build_program(meta):
    import concourse.bass as bass
    import concourse.mybir as mybir
    import concourse.tile as tile
    from concourse import bacc
    from concourse.masks import make_identity

    N = meta["N"]
    shard = meta["shard"]
    ngroups = meta["ngroups"]
    nsb = meta["nsb"]
    npad = meta["npad"]
    lo_cap = meta["lo_cap"]
    fTA, fTB, foffs = meta["fTA"], meta["fTB"], meta["foffs"]
    bTA, bTB, boffs = meta["bTA"], meta["bTB"], meta["boffs"]
    fTt = int(foffs[-1])
    bTt = int(boffs[-1])
    f16 = mybir.dt.float16
    f32 = mybir.dt.float32
    i16 = mybir.dt.int16

    nc = bacc.Bacc("TRN2", target_bir_lowering=False, debug=False,
                   num_devices=NCORES)

    # I/O
    x16_in = nc.dram_tensor("x16", [N, P], f16, kind="ExternalInput")
    xT0_in = nc.dram_tensor("xT0", [P, npad], f32, kind="ExternalInput")
    fidx_in = nc.dram_tensor("fidx", [P, 8 * fTt], i16, kind="ExternalInput")
    fdst_in = nc.dram_tensor("fdst", [P, fTt], f32, kind="ExternalInput")
    fval_in = nc.dram_tensor("fval", [P, fTt], f32, kind="ExternalInput")
    bidx_in = nc.dram_tensor("bidx", [P, 8 * bTt], i16, kind="ExternalInput")
    bdst_in = nc.dram_tensor("bdst", [P, bTt], f32, kind="ExternalInput")
    bval_in = nc.dram_tensor("bval", [P, bTt], f32, kind="ExternalInput")
    iota_in = nc.dram_tensor("iota", [P, GROUP], f16, kind="ExternalInput")
    W_in = {}
    for li in range(3):
        W_in[li] = (
            nc.dram_tensor(f"Wx{li}", [P, P], f32, kind="ExternalInput"),
            nc.dram_tensor(f"Wf{li}", [P, P], f16, kind="ExternalInput"),
            nc.dram_tensor(f"Wb{li}", [P, P], f16, kind="ExternalInput"),
            nc.dram_tensor(f"bias{li}", [P, 1], f32, kind="ExternalInput"),
        )
    y_out = nc.dram_tensor("y", [shard, P], f32, kind="ExternalOutput")

    with tile.TileContext(nc) as tc:
        from contextlib import ExitStack
        ctx = ExitStack()
        with ctx:
            const_pool = ctx.enter_context(tc.tile_pool(name="const", bufs=1))
            dram_pool = ctx.enter_context(
                tc.tile_pool(name="dram", bufs=1, space="DRAM"))
            gath_pool = ctx.enter_context(tc.tile_pool(name="gath", bufs=3))
            oh_pool = ctx.enter_context(tc.tile_pool(name="oh", bufs=4))
            agg_pool = ctx.enter_context(tc.tile_pool(name="aggsb", bufs=3))
            stg_pool = ctx.enter_context(tc.tile_pool(name="stg", bufs=2))
            ps_fwd = ctx.enter_context(
                tc.tile_pool(name="psfwd", bufs=2, space="PSUM"))
            ps_bwd = ctx.enter_context(
                tc.tile_pool(name="psbwd", bufs=2, space="PSUM"))
            ps_lin = ctx.enter_context(
                tc.tile_pool(name="pslin", bufs=2, space="PSUM"))
            ps_tr = ctx.enter_context(
                tc.tile_pool(name="pstr", bufs=2, space="PSUM"))

            # shared DRAM tables (AllGather outputs, one per layer boundary)
            # + local shards (AG inputs)
            table_ts = [
                dram_pool.tile([N, P], f16, addr_space="Shared",
                               name=f"table{i}", tag=f"table{i}")
                for i in range(2)
            ]
            shard_ts = [
                dram_pool.tile([shard, P], f16, name=f"shardbuf{i}",
                               tag=f"shardbuf{i}")
                for i in range(2)
            ]

            # resident constants
            def resident(name, dram, shape, dtype):
                t = const_pool.tile(shape, dtype, name=name)
                nc.sync.dma_start(t[:, :], dram[:, :])
                return t

            xT_a = resident("xT_a", xT0_in, [P, npad], f32)
            xT_b = const_pool.tile([P, npad], f32, name="xT_b")
            fidx_t = resident("fidx_t", fidx_in, [P, 8 * fTt], i16)
            fdst_t = resident("fdst_t", fdst_in, [P, fTt], f32)
            fval_t = resident("fval_t", fval_in, [P, fTt], f32)
            bidx_t = resident("bidx_t", bidx_in, [P, 8 * bTt], i16)
            bdst_t = resident("bdst_t", bdst_in, [P, bTt], f32)
            bval_t = resident("bval_t", bval_in, [P, bTt], f32)
            iota_t = resident("iota_t", iota_in, [P, GROUP], f16)
            Wt = {}
            for li in range(3):
                Wt[li] = (
                    resident(f"Wx{li}_t", W_in[li][0], [P, P], f32),
                    resident(f"Wf{li}_t", W_in[li][1], [P, P], f16),
                    resident(f"Wb{li}_t", W_in[li][2], [P, P], f16),
                    resident(f"bias{li}_t", W_in[li][3], [P, 1], f32),
                )
            ident_t = const_pool.tile([P, P], f32, name="ident_t")
            make_identity(nc, ident_t[:, :])

            def aggregate(direction, g, src_lo, src_hi, psum_ap):
                """Accumulate group g of one direction into psum_ap [P, GROUP]."""
                if direction == 0:
                    TA, TB, offs = fTA, fTB, foffs
                    idx_t, dst_t, val_t = fidx_t, fdst_t, fval_t
                else:
                    TA, TB, offs = bTA, bTB, boffs
                    idx_t, dst_t, val_t = bidx_t, bdst_t, bval_t
                col0 = int(offs[g])
                tA, tB = int(TA[g]), int(TB[g])
                total = tA + tB
                done = 0
                for half, tn, src_ap in ((0, tA, src_lo), (1, tB, src_hi)):
                    if tn == 0:
                        continue
                    c0 = col0 + (tA if half else 0)
                    num = tn * P
                    gbuf = gath_pool.tile([P, num], f16, tag="gath")
                    nc.gpsimd.dma_gather(
                        out_ap=gbuf[:, :].rearrange("p (t f) -> p t f", f=P),
                        in_ap=src_ap,
                        idxs_ap=idx_t[:, 8 * c0:8 * (c0 + tn)],
                        num_idxs=num,
                        num_idxs_reg=num,
                        elem_size=P,
                        single_packet=False,
                    )
                    for t in range(tn):
                        col = c0 + t
                        oh = oh_pool.tile([P, GROUP], f16, tag="oh")
                        nc.vector.tensor_scalar(
                            oh[:, :], iota_t[:, :],
                            dst_t[:, col:col + 1], val_t[:, col:col + 1],
                            op0=mybir.AluOpType.is_equal,
                            op1=mybir.AluOpType.mult,
                        )
                        nc.tensor.matmul(
                            psum_ap,
                            lhsT=gbuf[:, t * P:(t + 1) * P],
                            rhs=oh[:, :],
                            start=(done == 0),
                            stop=(done == total - 1),
                        )
                        done += 1

            for li in range(3):
                xT_cur = xT_a if li % 2 == 0 else xT_b
                xT_nxt = xT_b if li % 2 == 0 else xT_a
                src_tbl = x16_in if li == 0 else table_ts[li - 1]
                src_lo = src_tbl[0:min(lo_cap, N), :]
                src_hi = src_tbl[lo_cap:N, :] if lo_cap < N else None
                shard_t = shard_ts[li] if li < 2 else None
                Wx_t, Wf_t, Wb_t, bias_t = Wt[li]
                last = li == 2

                for s in range(nsb):
                    glist = [g for g in (2 * s, 2 * s + 1) if g < ngroups]
                    fwd_ps = ps_fwd.tile([P, SB], f32, tag="psf")
                    bwd_ps = ps_bwd.tile([P, SB], f32, tag="psb")
                    for g in glist:
                        half = (g % 2) * GROUP
                        aggregate(0, g, src_lo, src_hi,
                                  fwd_ps[:, half:half + GROUP])
                        aggregate(1, g, src_lo, src_hi,
                                  bwd_ps[:, half:half + GROUP])
                    if len(glist) == 1:
                        # initialize unused psum half so downstream reads are
                        # defined (columns are discarded later)
                        nc.vector.memset(fwd_ps[:, GROUP:], 0.0)
                        nc.vector.memset(bwd_ps[:, GROUP:], 0.0)

                    fwd_sb = agg_pool.tile([P, SB], f16, tag="aggf")
                    bwd_sb = agg_pool.tile([P, SB], f16, tag="aggb")
                    nc.scalar.activation(fwd_sb[:, :], fwd_ps[:, :],
                                         mybir.ActivationFunctionType.Copy)
                    nc.scalar.activation(bwd_sb[:, :], bwd_ps[:, :],
                                         mybir.ActivationFunctionType.Copy)

                    lin_ps = ps_lin.tile([P, SB], f32, tag="psl")
                    n0 = s * SB
                    nc.tensor.matmul(lin_ps[:, :], lhsT=Wx_t[:, :],
                                     rhs=xT_cur[:, n0:n0 + SB],
                                     start=True, stop=False)
                    nc.tensor.matmul(lin_ps[:, :], lhsT=Wf_t[:, :],
                                     rhs=fwd_sb[:, :], start=False, stop=False)
                    nc.tensor.matmul(lin_ps[:, :], lhsT=Wb_t[:, :],
                                     rhs=bwd_sb[:, :], start=False, stop=True)

                    if not last:
                        # bias + relu -> next-layer features (feature-major)
                        nc.scalar.activation(
                            xT_nxt[:, n0:n0 + SB], lin_ps[:, :],
                            mybir.ActivationFunctionType.Relu,
                            bias=bias_t[:, 0:1])
                        # transpose to node-major fp16, stage, write shard
                        tr_ps = ps_tr.tile([P, SB], f32, tag="pst")
                        stg = stg_pool.tile([P, SB], f16, tag="stg16")
                        for k in range(SB // P):
                            c0 = n0 + k * P
                            nc.tensor.transpose(
                                tr_ps[:, k * P:(k + 1) * P],
                                xT_nxt[:, c0:c0 + P], ident_t[:, :])
                        nc.scalar.activation(stg[:, :], tr_ps[:, :],
                                             mybir.ActivationFunctionType.Copy)
                        valid = min(SB, shard - n0)
                        kf = valid // P
                        rem = valid % P
                        if kf > 0:
                            out_ap = shard_t[n0:n0 + kf * P, :].rearrange(
                                "(k p) f -> p k f", p=P)
                            in_ap = stg[:, 0:kf * P].rearrange(
                                "p (k f) -> p k f", f=P)
                            nc.sync.dma_start(out_ap, in_ap)
                        if rem > 0:
                            nc.sync.dma_start(
                                shard_t[n0 + kf * P:n0 + valid, :],
                                stg[0:rem, kf * P:(kf + 1) * P])
                    else:
                        # bias only, fp32, write final output
                        outT = agg_pool.tile([P, SB], f32, tag="outT")
                        nc.scalar.activation(
                            outT[:, :], lin_ps[:, :],
                            mybir.ActivationFunctionType.Identity,
                            bias=bias_t[:, 0:1])
                        tr_ps = ps_tr.tile([P, SB], f32, tag="pst")
                        stg32 = stg_pool.tile([P, SB], f32, tag="stg32")
                        for k in range(SB // P):
                            nc.tensor.transpose(
                                tr_ps[:, k * P:(k + 1) * P],
                                outT[:, k * P:(k + 1) * P], ident_t[:, :])
                        nc.scalar.activation(stg32[:, :], tr_ps[:, :],
                                             mybir.ActivationFunctionType.Copy)
                        valid = min(SB, shard - n0)
                        kf = valid // P
                        rem = valid % P
                        if kf > 0:
                            out_ap = y_out[n0:n0 + kf * P, :].rearrange(
                                "(k p) f -> p k f", p=P)
                            in_ap = stg32[:, 0:kf * P].rearrange(
                                "p (k f) -> p k f", f=P)
                            nc.sync.dma_start(out_ap, in_ap)
                        if rem > 0:
                            nc.sync.dma_start(
                                y_out[n0 + kf * P:n0 + valid, :],
                                stg32[0:rem, kf * P:(kf + 1) * P])

                if not last and SKIP_COLLECTIVE:
                    # debug mode: fake the table update with a local copy
                    # (numerically wrong across cores, structurally similar)
                    nc.sync.dma_start(
                        table_ts[li][0:shard, :], shard_t[:, :])
                elif not last:
                    nc.gpsimd.collective_compute(
                        "AllGather",
                        mybir.AluOpType.bypass,
                        replica_groups=[list(range(NCORES))],
                        ins=[shard_t[:, :]],
                        outs=[table_ts[li][:, :]],
                    )
    nc.compile()
    return nc


# ----------------------------------------------------------------------------
# Entry point
# ----------------------------------------------------------------------------

LO_CAP = 32768  # int16 gather-index limit; tests may lower this


def kernel(x, W0, b0, W1, b1, W2, b2, src, dst):
    x = np.asarray(x, dtype=np.float32)
    src = np.asarray(src, dtype=np.int32)
    dst = np.asarray(dst, dtype=np.int32)
    W0, b0 = np.asarray(W0, np.float32), np.asarray(b0, np.float32)
    W1, b1 = np.asarray(W1, np.float32), np.asarray(b1, np.float32)
    W2, b2 = np.asarray(W2, np.float32), np.asarray(b2, np.float32)

    lo_cap = min(LO_CAP, x.shape[0])
    global _CACHE
    ck = (x.shape, src.shape, lo_cap, float(x[0, 0]), int(src[0]),
          int(dst[0]))
    if _CACHE is not None and _CACHE[0] == ck:
        meta, per_core, nc = _CACHE[1]
    else:
        meta, per_core = _host_prep(x, W0, b0, W1, b1, W2, b2, src, dst,
                                    lo_cap)
        nc = _build_program(meta)
        _CACHE = (ck, (meta, per_core, nc))

    from concourse.bass_utils import run_bass_kernel_spmd
    trace = os.environ.get("KERNEL_TRACE", "0") == "1"
    res = run_bass_kernel_spmd(nc, per_core, core_ids=list(range(NCORES)),
                               trace=trace)
    global LAST_EXEC_NS
    LAST_EXEC_NS = res.exec_time_ns
    shard = meta["shard"]
    out = np.concatenate([res.results[c]["y"] for c in range(NCORES)], axis=0)
    assert out.shape == x.shape
    return out.astype(np.float32)


LAST_EXEC_NS = None
_CACHE = None


# revision 25
# speedup vs baseline: 1.0134x; 1.0134x over previous
"""Directional GraphSAGE (3-layer) Trainium2 kernel, 8-core SPMD.

Strategy:
  - Nodes sharded contiguously across 8 cores (N/8 per core).
  - fwd aggregation (segment-sum over dst) partitioned by dst owner;
    bwd aggregation (segment-sum over src) partitioned by src owner.
  - Per core, its node range is split into 256-node GROUPS. Each group's
    incoming edges are gathered (fp16 rows, indirect DMA from an HBM table)
    in 128-edge tiles; a scaled one-hot matrix [128 edges x 256 dst] is built
    on the vector engine (one tensor_scalar: (iota == dst_local) * val) and
    the TensorEngine accumulates  msgs^T @ onehot  ->  PSUM [128 feat, 256]
    (feature-major aggregation, fp32 accumulate).
  - Linear layer: out_T[fo, n] = Wx^T x_T + Wf^T fwd_T + Wb^T bwd_T in PSUM,
    bias+ReLU on the scalar engine. Everything stays feature-major; the x
    (self) path stays fp32 end-to-end.
  - New features are PE-transposed to node-major fp16 rows, DMA'd to a
    per-core shard buffer, and AllGather'd into the next layer's gather table.
  - All data-dependent constants (per-group tile counts) are maxed across
    cores so the single SPMD program is structurally identical on all cores;
    per-core differences live in input arrays (gather indices, dst slots,
    edge values).
"""

import os
import sys

import numpy as np

sys.path.insert(0, "/opt/trn_rl_repo")

NCORES = int(os.environ.get("KERNEL_NCORES", "8"))
SKIP_COLLECTIVE = os.environ.get("KERNEL_SKIP_COLLECTIVE", "0") == "1"
P = 128            # partitions / feature dim tile
GROUP = 256        # dst nodes covered by one PSUM accumulation group
SB = 512           # nodes per superblock (2 groups)


# ----------------------------------------------------------------------------
# Host-side prep
# ----------------------------------------------------------------------------

def _prep_direction(gather_node, out_node, val, N, shard, lo_cap):
    """Build per-core gather/onehot grids for one aggregation direction.

    For edge e: contributes val[e] * x[gather_node[e]] to output row
    out_node[e].  Partitioned by out_node's owning core; within a core,
    grouped by 256-node groups of the local node range; within a group,
    split into lo (gather row < lo_cap) and hi halves so int16 gather
    indices stay in range.

    Tile layout per (group, half): edge slot i -> (partition i%128,
    tile i//128), matching dma_gather's output placement.  dst/val grids
    [128, sum(T)] hold per-tile onehot scalars (column = one tile); the
    idx grid [128, 8*sum(T)] holds int16 indices in dma_gather's
    16-partition-wrapped layout replicated across the 8 Q7 cores.
    """
    ngroups = (shard + GROUP - 1) // GROUP
    owner = out_node // shard
    local = out_node - owner * shard
    group = local // GROUP
    dst_local = (local - group * GROUP).astype(np.float32)
    is_hi = gather_node >= lo_cap

    # counts[c, g, half]
    counts = np.zeros((NCORES, ngroups, 2), dtype=np.int64)
    np.add.at(counts, (owner, group, is_hi.astype(np.int64)), 1)
    cmax = counts.max(axis=0)  # [ngroups, 2]
    TA = ((cmax[:, 0] + P - 1) // P).astype(np.int64)
    TB = ((cmax[:, 1] + P - 1) // P).astype(np.int64)
    TA = np.maximum(TA, (TB == 0).astype(np.int64))  # >=1 tile per group
    Tsum = TA + TB
    offs = np.concatenate([[0], np.cumsum(Tsum)]).astype(np.int64)
    Tt = int(offs[-1])

    idx_g = np.zeros((NCORES, P, 8 * Tt), dtype=np.int16)
    dst_g = np.zeros((NCORES, P, Tt), dtype=np.float32)
    val_g = np.zeros((NCORES, P, Tt), dtype=np.float32)

    # sort edges by (owner, group, half) once
    key = (owner * ngroups + group) * 2 + is_hi
    order = np.argsort(key, kind="stable")
    ks = key[order]
    gn = gather_node[order]
    dl = dst_local[order]
    vv = val[order]
    nkey = NCORES * ngroups * 2
    starts = np.searchsorted(ks, np.arange(nkey))
    ends = np.searchsorted(ks, np.arange(nkey) + 1)

    def fill(c, col0, t, rows, dlocal, vals):
        """Place a padded run of t tiles at dv-columns [col0, col0+t)."""
        cap = P * t
        n = len(rows)
        assert n <= cap
        bi = np.zeros(cap, dtype=np.int16)
        bd = np.zeros(cap, dtype=np.float32)
        bv = np.zeros(cap, dtype=np.float32)
        bi[:n] = rows.astype(np.int16)
        bd[:n] = dlocal
        bv[:n] = vals
        dst_g[c, :, col0:col0 + t] = bd.reshape(t, P).T
        val_g[c, :, col0:col0 + t] = bv.reshape(t, P).T
        # wrapped idx: element j at (j%16, j//16), replicated x8
        wrapped = bi.reshape(cap // 16, 16).T  # [16, 8t]
        idx_g[c, :, 8 * col0:8 * (col0 + t)] = np.tile(wrapped, (8, 1))

    for c in range(NCORES):
        for g in range(ngroups):
            kbase = (c * ngroups + g) * 2
            sA, eA = starts[kbase], ends[kbase]
            sB, eB = starts[kbase + 1], ends[kbase + 1]
            tA, tB = int(TA[g]), int(TB[g])
            col0 = int(offs[g])
            if tA > 0:
                fill(c, col0, tA, gn[sA:eA], dl[sA:eA], vv[sA:eA])
            if tB > 0:
                fill(c, col0 + tA, tB, gn[sB:eB] - lo_cap, dl[sB:eB],
                     vv[sB:eB])
    return TA, TB, offs, idx_g, dst_g, val_g


def _host_prep(x, W0, b0, W1, b1, W2, b2, src, dst, lo_cap):
    N, D = x.shape
    assert D == P
    assert N % NCORES == 0, N
    shard = N // NCORES

    deg_out = np.bincount(src, minlength=N).astype(np.float32)
    deg_in = np.bincount(dst, minlength=N).astype(np.float32)
    inv_out = 1.0 / np.maximum(deg_out, 1.0)
    inv_in = 1.0 / np.maximum(deg_in, 1.0)

    # fwd: out row = dst, gather src, scale inv_out[src]
    fTA, fTB, foffs, fidx, fdst, fval = _prep_direction(
        src.astype(np.int64), dst.astype(np.int64), inv_out[src], N, shard,
        lo_cap)
    # bwd: out row = src, gather dst, scale inv_in[dst]
    bTA, bTB, boffs, bidx, bdst, bval = _prep_direction(
        dst.astype(np.int64), src.astype(np.int64), inv_in[dst], N, shard,
        lo_cap)

    ngroups = (shard + GROUP - 1) // GROUP
    nsb = (ngroups + 1) // 2
    npad = nsb * SB

    x16 = np.ascontiguousarray(x.astype(np.float16))
    xT = np.zeros((NCORES, P, npad), dtype=np.float32)
    for c in range(NCORES):
        xT[c, :, :shard] = x[c * shard:(c + 1) * shard].T

    iota = np.tile(np.arange(GROUP, dtype=np.float16), (P, 1))

    Ws, bs = [], []
    for W, b in ((W0, b0), (W1, b1), (W2, b2)):
        Ws.append((np.ascontiguousarray(W[0:P]).astype(np.float32),
                   np.ascontiguousarray(W[P:2 * P]).astype(np.float16),
                   np.ascontiguousarray(W[2 * P:3 * P]).astype(np.float16)))
        bs.append(b.reshape(P, 1).astype(np.float32))

    meta = dict(N=N, shard=shard, ngroups=ngroups, nsb=nsb, npad=npad,
                lo_cap=lo_cap, fTA=fTA, fTB=fTB, foffs=foffs,
                bTA=bTA, bTB=bTB, boffs=boffs)
    per_core = []
    for c in range(NCORES):
        m = {
            "x16": x16,
            "xT0": np.ascontiguousarray(xT[c]),
            "fidx": np.ascontiguousarray(fidx[c]),
            "fdst": np.ascontiguousarray(fdst[c]),
            "fval": np.ascontiguousarray(fval[c]),
            "bidx": np.ascontiguousarray(bidx[c]),
            "bdst": np.ascontiguousarray(bdst[c]),
            "bval": np.ascontiguousarray(bval[c]),
            "iota": iota,
        }
        for li in range(3):
            m[f"Wx{li}"] = Ws[li][0]
            m[f"Wf{li}"] = Ws[li][1]
            m[f"Wb{li}"] = Ws[li][2]
            m[f"bias{li}"] = bs[li]
        per_core.append(m)
    return meta, per_core


# ----------------------------------------------------------------------------
# Bass program
# ----------------------------------------------------------------------------

def _build_program(meta):
    import concourse.bass as bass
    import concourse.mybir as mybir
    import concourse.tile as tile
    from concourse import bacc
    from concourse.masks import make_identity

    N = meta["N"]
    shard = meta["shard"]
    ngroups = meta["ngroups"]
    nsb = meta["nsb"]
    npad = meta["npad"]
    lo_cap = meta["lo_cap"]
    fTA, fTB, foffs = meta["fTA"], meta["fTB"], meta["foffs"]
    bTA, bTB, boffs = meta["bTA"], meta["bTB"], meta["boffs"]
    fTt = int(foffs[-1])
    bTt = int(boffs[-1])
    f16 = mybir.dt.float16
    f32 = mybir.dt.float32
    i16 = mybir.dt.int16

    nc = bacc.Bacc("TRN2", target_bir_lowering=False, debug=False,
                   num_devices=NCORES, num_swdge_queues=4)

    # I/O
    x16_in = nc.dram_tensor("x16", [N, P], f16, kind="ExternalInput")
    xT0_in = nc.dram_tensor("xT0", [P, npad], f32, kind="ExternalInput")
    fidx_in = nc.dram_tensor("fidx", [P, 8 * fTt], i16, kind="ExternalInput")
    fdst_in = nc.dram_tensor("fdst", [P, fTt], f32, kind="ExternalInput")
    fval_in = nc.dram_tensor("fval", [P, fTt], f32, kind="ExternalInput")
    bidx_in = nc.dram_tensor("bidx", [P, 8 * bTt], i16, kind="ExternalInput")
    bdst_in = nc.dram_tensor("bdst", [P, bTt], f32, kind="ExternalInput")
    bval_in = nc.dram_tensor("bval", [P, bTt], f32, kind="ExternalInput")
    iota_in = nc.dram_tensor("iota", [P, GROUP], f16, kind="ExternalInput")
    W_in = {}
    for li in range(3):
        W_in[li] = (
            nc.dram_tensor(f"Wx{li}", [P, P], f32, kind="ExternalInput"),
            nc.dram_tensor(f"Wf{li}", [P, P], f16, kind="ExternalInput"),
            nc.dram_tensor(f"Wb{li}", [P, P], f16, kind="ExternalInput"),
            nc.dram_tensor(f"bias{li}", [P, 1], f32, kind="ExternalInput"),
        )
    y_out = nc.dram_tensor("y", [shard, P], f32, kind="ExternalOutput")

    with tile.TileContext(nc) as tc:
        from contextlib import ExitStack
        ctx = ExitStack()
        with ctx:
            const_pool = ctx.enter_context(tc.tile_pool(name="const", bufs=1))
            dram_pool = ctx.enter_context(
                tc.tile_pool(name="dram", bufs=1, space="DRAM"))
            gath_pool = ctx.enter_context(tc.tile_pool(name="gath", bufs=3))
            oh_pool = ctx.enter_context(tc.tile_pool(name="oh", bufs=4))
            agg_pool = ctx.enter_context(tc.tile_pool(name="aggsb", bufs=3))
            stg_pool = ctx.enter_context(tc.tile_pool(name="stg", bufs=2))
            ps_fwd = ctx.enter_context(
                tc.tile_pool(name="psfwd", bufs=2, space="PSUM"))
            ps_bwd = ctx.enter_context(
                tc.tile_pool(name="psbwd", bufs=2, space="PSUM"))
            ps_lin = ctx.enter_context(
                tc.tile_pool(name="pslin", bufs=2, space="PSUM"))
            ps_tr = ctx.enter_context(
                tc.tile_pool(name="pstr", bufs=2, space="PSUM"))

            # shared DRAM tables (AllGather outputs, one per layer boundary)
            # + local shards (AG inputs)
            table_ts = [
                dram_pool.tile([N, P], f16, addr_space="Shared",
                               name=f"table{i}", tag=f"table{i}")
                for i in range(2)
            ]
            shard_ts = [
                dram_pool.tile([shard, P], f16, name=f"shardbuf{i}",
                               tag=f"shardbuf{i}")
                for i in range(2)
            ]

            # resident constants
            def resident(name, dram, shape, dtype):
                t = const_pool.tile(shape, dtype, name=name)
                nc.sync.dma_start(t[:, :], dram[:, :])
                return t

            xT_a = resident("xT_a", xT0_in, [P, npad], f32)
            xT_b = const_pool.tile([P, npad], f32, name="xT_b")
            fidx_t = resident("fidx_t", fidx_in, [P, 8 * fTt], i16)
            fdst_t = resident("fdst_t", fdst_in, [P, fTt], f32)
            fval_t = resident("fval_t", fval_in, [P, fTt], f32)
            bidx_t = resident("bidx_t", bidx_in, [P, 8 * bTt], i16)
            bdst_t = resident("bdst_t", bdst_in, [P, bTt], f32)
            bval_t = resident("bval_t", bval_in, [P, bTt], f32)
            iota_t = resident("iota_t", iota_in, [P, GROUP], f16)
            Wt = {}
            for li in range(3):
                Wt[li] = (
                    resident(f"Wx{li}_t", W_in[li][0], [P, P], f32),
                    resident(f"Wf{li}_t", W_in[li][1], [P, P], f16),
                    resident(f"Wb{li}_t", W_in[li][2], [P, P], f16),
                    resident(f"bias{li}_t", W_in[li][3], [P, 1], f32),
                )
            ident_t = const_pool.tile([P, P], f32, name="ident_t")
            make_identity(nc, ident_t[:, :])

            gather_seq = [0]  # round-robin SWDGE queue assignment

            def aggregate(direction, g, src_lo, src_hi, psum_ap):
                """Accumulate group g of one direction into psum_ap [P, GROUP]."""
                if direction == 0:
                    TA, TB, offs = fTA, fTB, foffs
                    idx_t, dst_t, val_t = fidx_t, fdst_t, fval_t
                else:
                    TA, TB, offs = bTA, bTB, boffs
                    idx_t, dst_t, val_t = bidx_t, bdst_t, bval_t
                col0 = int(offs[g])
                tA, tB = int(TA[g]), int(TB[g])
                total = tA + tB
                done = 0
                for half, tn, src_ap in ((0, tA, src_lo), (1, tB, src_hi)):
                    if tn == 0:
                        continue
                    c0 = col0 + (tA if half else 0)
                    num = tn * P
                    gbuf = gath_pool.tile([P, num], f16, tag="gath")
                    qn = gather_seq[0] % 4
                    gather_seq[0] += 1
                    nc.gpsimd.dma_gather(
                        out_ap=gbuf[:, :].rearrange("p (t f) -> p t f", f=P),
                        in_ap=src_ap,
                        idxs_ap=idx_t[:, 8 * c0:8 * (c0 + tn)],
                        num_idxs=num,
                        num_idxs_reg=num,
                        elem_size=P,
                        single_packet=False,
                        queue_num=qn,
                    )
                    for t in range(tn):
                        col = c0 + t
                        oh = oh_pool.tile([P, GROUP], f16, tag="oh")
                        nc.vector.tensor_scalar(
                            oh[:, :], iota_t[:, :],
                            dst_t[:, col:col + 1], val_t[:, col:col + 1],
                            op0=mybir.AluOpType.is_equal,
                            op1=mybir.AluOpType.mult,
                        )
                        nc.tensor.matmul(
                            psum_ap,
                            lhsT=gbuf[:, t * P:(t + 1) * P],
                            rhs=oh[:, :],
                            start=(done == 0),
                            stop=(done == total - 1),
                        )
                        done += 1

            for li in range(3):
                xT_cur = xT_a if li % 2 == 0 else xT_b
                xT_nxt = xT_b if li % 2 == 0 else xT_a
                src_tbl = x16_in if li == 0 else table_ts[li - 1]
                src_lo = src_tbl[0:min(lo_cap, N), :]
                src_hi = src_tbl[lo_cap:N, :] if lo_cap < N else None
                shard_t = shard_ts[li] if li < 2 else None
                Wx_t, Wf_t, Wb_t, bias_t = Wt[li]
                last = li == 2

                for s in range(nsb):
                    glist = [g for g in (2 * s, 2 * s + 1) if g < ngroups]
                    fwd_ps = ps_fwd.tile([P, SB], f32, tag="psf")
                    bwd_ps = ps_bwd.tile([P, SB], f32, tag="psb")
                    for g in glist:
                        half = (g % 2) * GROUP
                        aggregate(0, g, src_lo, src_hi,
                                  fwd_ps[:, half:half + GROUP])
                        aggregate(1, g, src_lo, src_hi,
                                  bwd_ps[:, half:half + GROUP])
                    if len(glist) == 1:
                        # initialize unused psum half so downstream reads are
                        # defined (columns are discarded later)
                        nc.vector.memset(fwd_ps[:, GROUP:], 0.0)
                        nc.vector.memset(bwd_ps[:, GROUP:], 0.0)

                    fwd_sb = agg_pool.tile([P, SB], f16, tag="aggf")
                    bwd_sb = agg_pool.tile([P, SB], f16, tag="aggb")
                    nc.scalar.activation(fwd_sb[:, :], fwd_ps[:, :],
                                         mybir.ActivationFunctionType.Copy)
                    nc.scalar.activation(bwd_sb[:, :], bwd_ps[:, :],
                                         mybir.ActivationFunctionType.Copy)

                    lin_ps = ps_lin.tile([P, SB], f32, tag="psl")
                    n0 = s * SB
                    nc.tensor.matmul(lin_ps[:, :], lhsT=Wx_t[:, :],
                                     rhs=xT_cur[:, n0:n0 + SB],
                                     start=True, stop=False)
                    nc.tensor.matmul(lin_ps[:, :], lhsT=Wf_t[:, :],
                                     rhs=fwd_sb[:, :], start=False, stop=False)
                    nc.tensor.matmul(lin_ps[:, :], lhsT=Wb_t[:, :],
                                     rhs=bwd_sb[:, :], start=False, stop=True)

                    if not last:
                        # bias + relu -> next-layer features (feature-major)
                        nc.scalar.activation(
                            xT_nxt[:, n0:n0 + SB], lin_ps[:, :],
                            mybir.ActivationFunctionType.Relu,
                            bias=bias_t[:, 0:1])
                        # transpose to node-major fp16, stage, write shard
                        tr_ps = ps_tr.tile([P, SB], f32, tag="pst")
                        stg = stg_pool.tile([P, SB], f16, tag="stg16")
                        for k in range(SB // P):
                            c0 = n0 + k * P
                            nc.tensor.transpose(
                                tr_ps[:, k * P:(k + 1) * P],
                                xT_nxt[:, c0:c0 + P], ident_t[:, :])
                        nc.scalar.activation(stg[:, :], tr_ps[:, :],
                                             mybir.ActivationFunctionType.Copy)
                        valid = min(SB, shard - n0)
                        kf = valid // P
                        rem = valid % P
                        if kf > 0:
                            out_ap = shard_t[n0:n0 + kf * P, :].rearrange(
                                "(k p) f -> p k f", p=P)
                            in_ap = stg[:, 0:kf * P].rearrange(
                                "p (k f) -> p k f", f=P)
                            nc.sync.dma_start(out_ap, in_ap)
                        if rem > 0:
                            nc.sync.dma_start(
                                shard_t[n0 + kf * P:n0 + valid, :],
                                stg[0:rem, kf * P:(kf + 1) * P])
                    else:
                        # bias only, fp32, write final output
                        outT = agg_pool.tile([P, SB], f32, tag="outT")
                        nc.scalar.activation(
                            outT[:, :], lin_ps[:, :],
                            mybir.ActivationFunctionType.Identity,
                            bias=bias_t[:, 0:1])
                        tr_ps = ps_tr.tile([P, SB], f32, tag="pst")
                        stg32 = stg_pool.tile([P, SB], f32, tag="stg32")
                        for k in range(SB // P):
                            nc.tensor.transpose(
                                tr_ps[:, k * P:(k + 1) * P],
                                outT[:, k * P:(k + 1) * P], ident_t[:, :])
                        nc.scalar.activation(stg32[:, :], tr_ps[:, :],
                                             mybir.ActivationFunctionType.Copy)
                        valid = min(SB, shard - n0)
                        kf = valid // P
                        rem = valid % P
                        if kf > 0:
                            out_ap = y_out[n0:n0 + kf * P, :].rearrange(
                                "(k p) f -> p k f", p=P)
                            in_ap = stg32[:, 0:kf * P].rearrange(
                                "p (k f) -> p k f", f=P)
                            nc.sync.dma_start(out_ap, in_ap)
                        if rem > 0:
                            nc.sync.dma_start(
                                y_out[n0 + kf * P:n0 + valid, :],
                                stg32[0:rem, kf * P:(kf + 1) * P])

                if not last and SKIP_COLLECTIVE:
                    # debug mode: fake the table update with a local copy
                    # (numerically wrong across cores, structurally similar)
                    nc.sync.dma_start(
                        table_ts[li][0:shard, :], shard_t[:, :])
                elif not last:
                    nc.gpsimd.collective_compute(
                        "AllGather",
                        mybir.AluOpType.bypass,
                        replica_groups=[list(range(NCORES))],
                        ins=[shard_t[:, :]],
                        outs=[table_ts[li][:, :]],
                    )
    nc.compile()
    return nc


# ----------------------------------------------------------------------------
# Entry point
# ----------------------------------------------------------------------------

LO_CAP = 32768  # int16 gather-index limit; tests may lower this


def kernel(x, W0, b0, W1, b1, W2, b2, src, dst):
    x = np.asarray(x, dtype=np.float32)
    src = np.asarray(src, dtype=np.int32)
    dst = np.asarray(dst, dtype=np.int32)
    W0, b0 = np.asarray(W0, np.float32), np.asarray(b0, np.float32)
    W1, b1 = np.asarray(W1, np.float32), np.asarray(b1, np.float32)
    W2, b2 = np.asarray(W2, np.float32), np.asarray(b2, np.float32)

    lo_cap = min(LO_CAP, x.shape[0])
    global _CACHE
    ck = (x.shape, src.shape, lo_cap, float(x[0, 0]), int(src[0]),
          int(dst[0]))
    if _CACHE is not None and _CACHE[0] == ck:
        meta, per_core, nc = _CACHE[1]
    else:
        meta, per_core = _host_prep(x, W0, b0, W1, b1, W2, b2, src, dst,
                                    lo_cap)
        nc = _build_program(meta)
        _CACHE = (ck, (meta, per_core, nc))

    from concourse.bass_utils import run_bass_kernel_spmd
    trace = os.environ.get("KERNEL_TRACE", "0") == "1"
    res = run_bass_kernel_spmd(nc, per_core, core_ids=list(range(NCORES)),
                               trace=trace)
    global LAST_EXEC_NS
    LAST_EXEC_NS = res.exec_time_ns
    shard = meta["shard"]
    out = np.concatenate([res.results[c]["y"] for c in range(NCORES)], axis=0)
    assert out.shape == x.shape
    return out.astype(np.float32)


LAST_EXEC_NS = None
_CACHE = None
